# revision 2
# baseline (speedup 1.0000x reference)
"""Trainium2 Bass kernel: InterpretableMultiHeadAttention (shared-V MHA).

Contract: kernel(x, w_qkv, w_out) -> (out, attn_prob), full (unsharded) numpy
in/out. Internally: data-parallel over batch across 8 NeuronCores (2 batches
per core), bf16 compute on TensorE, f32 softmax/outputs.

Per-core dataflow (B=2 local batches, T=1024, H=1024, NH=4, DH=256):
  - host pre-transposes x to xT[b, h, t] bf16 (avoids on-chip transposes;
    q-scale 1/sqrt(DH) folded into w_q, head-mean 1/NH folded into w_out)
  - qkT[f, t] = w_qk.T @ x.T   (lhsT = w_qk tiles, rhs = xT tiles)
  - v[s, d]   = x @ w_v        (lhsT = xT tiles,   rhs = w_v tiles)
  - per head: scores[t, s] psum -> +tri mask on diag tile -> Exp (ACT, with
    row-sum accum) -> reciprocal -> normalize -> f32 DMA to attn_prob
    (upper-triangular tiles left unwritten; PJRT pre-zeros output buffers)
    -> bf16 cast -> per-tile SBUF->SBUF DMA transpose into pT[s, t]
  - attnV: psum[d, t] += v[s,:].T @ pT[s, t] accumulated over s AND heads
    (head-sum free in PSUM; suffix-N per s-tile skips masked wedge)
  - out[t, :] = (sum_heads attn_vec).T @ w_out (1/NH prefolded)
"""

import numpy as np

N_HEAD = 4
HIDDEN = 1024
D_HEAD = HIDDEN // N_HEAD  # 256
SEQ = 1024
BATCH = 16
N_CORES = 8
BPC = BATCH // N_CORES  # batches per core

P = 128
TBLK = 512  # matmul moving free-dim block

_compiled = {}


def _chunks(total, size):
    out = []
    o = 0
    while o < total:
        out.append((o, min(size, total - o)))
        o += size
    return out


def _emit(tc, aps, B, T, H, NH, DH):
    import concourse.mybir as mybir

    nc = tc.nc
    bf16 = mybir.dt.bfloat16
    f32 = mybir.dt.float32
    Exp = mybir.ActivationFunctionType.Exp

    xT_ap, wqk_ap, wv_ap, wo_ap, tri_ap, out_ap, ap_ap = aps
    HT = H // P            # h-tiles over hidden
    FQK = 2 * NH * DH      # q+k features
    FT = FQK // P          # qk f-tiles
    NT = T // P            # t/s tiles
    DT = DH // P           # d-tiles per head
    tblocks = _chunks(T, TBLK)

    from contextlib import ExitStack
    ctx = ExitStack()
    const = ctx.enter_context(tc.tile_pool(name="const", bufs=1))
    xt_pool = ctx.enter_context(tc.tile_pool(name="xt", bufs=2))
    qk_pool = ctx.enter_context(tc.tile_pool(name="qkp", bufs=1))
    v_pool = ctx.enter_context(tc.tile_pool(name="vp", bufs=2))
    pt_pool = ctx.enter_context(tc.tile_pool(name="pt", bufs=2))
    p_pool = ctx.enter_context(tc.tile_pool(name="pp", bufs=3))
    pbf_pool = ctx.enter_context(tc.tile_pool(name="pbf", bufs=3))
    av_pool = ctx.enter_context(tc.tile_pool(name="avs", bufs=2))
    fin_pool = ctx.enter_context(tc.tile_pool(name="fin", bufs=3))
    st_pool = ctx.enter_context(tc.tile_pool(name="st", bufs=8))
    psA = ctx.enter_context(tc.tile_pool(name="psA", bufs=4, space="PSUM"))
    psB = ctx.enter_context(tc.tile_pool(name="psB", bufs=4, space="PSUM"))

    # ---- constants (loaded once) ----
    w_sb = const.tile([P, HT * FQK], bf16, tag="wqk")
    for h in range(HT):
        nc.sync.dma_start(w_sb[:, h * FQK:(h + 1) * FQK], wqk_ap[h * P:(h + 1) * P, :])
    wv_sb = const.tile([P, HT * DH], bf16, tag="wv")
    for h in range(HT):
        nc.sync.dma_start(wv_sb[:, h * DH:(h + 1) * DH], wv_ap[h * P:(h + 1) * P, :])
    wo_sb = const.tile([P, DT * H], bf16, tag="wo")
    for d in range(DT):
        nc.sync.dma_start(wo_sb[:, d * H:(d + 1) * H], wo_ap[d * P:(d + 1) * P, :])
    tri_sb = const.tile([P, P], f32, tag="tri")
    nc.sync.dma_start(tri_sb[:], tri_ap[:, :])

    for b in range(B):
        # ---- load xT (bf16) ----
        xt_sb = xt_pool.tile([P, HT * T], bf16, tag="xt")
        for h in range(HT):
            nc.sync.dma_start(xt_sb[:, h * T:(h + 1) * T],
                              xT_ap[b, h * P:(h + 1) * P, :])

        # ---- QK projection: qkT[f, t] ----
        qk_sb = qk_pool.tile([P, FT * T], bf16, tag="qk")
        for f in range(FT):
            for (t0, tn) in tblocks:
                ps = psA.tile([P, TBLK], f32, tag="ps")
                for h in range(HT):
                    nc.tensor.matmul(
                        ps[:, :tn],
                        w_sb[:, h * FQK + f * P: h * FQK + (f + 1) * P],
                        xt_sb[:, h * T + t0: h * T + t0 + tn],
                        start=(h == 0), stop=(h == HT - 1))
                nc.vector.tensor_copy(qk_sb[:, f * T + t0: f * T + t0 + tn],
                                      ps[:, :tn])

        # ---- V projection: v[s, d] ----
        v_sb = v_pool.tile([P, NT * DH], bf16, tag="v")
        for s in range(NT):
            ps = psA.tile([P, TBLK], f32, tag="ps")
            for h in range(HT):
                nc.tensor.matmul(
                    ps[:, :DH],
                    xt_sb[:, h * T + s * P: h * T + (s + 1) * P],
                    wv_sb[:, h * DH:(h + 1) * DH],
                    start=(h == 0), stop=(h == HT - 1))
            nc.vector.tensor_copy(v_sb[:, s * DH:(s + 1) * DH], ps[:, :DH])

        # ---- attention ----
        avps = [psB.tile([P, TBLK], f32, tag="av", name=f"avp{i}")
                for i in range(DT * len(tblocks))]
        for n in range(NH):
            pt_sb = pt_pool.tile([P, NT * T], bf16, tag="pt")
            for tau in range(NT):
                ext = (tau + 1) * P
                p_sb = p_pool.tile([P, T], f32, tag="p")
                pbf_sb = pbf_pool.tile([P, T], bf16, tag="pb")
                schunks = _chunks(ext, TBLK)
                parts = []
                for ci, (c0, cn) in enumerate(schunks):
                    ps = psA.tile([P, TBLK], f32, tag="ps")
                    for d in range(DT):
                        nc.tensor.matmul(
                            ps[:, :cn],
                            qk_sb[:, (n * DT + d) * T + tau * P:
                                  (n * DT + d) * T + (tau + 1) * P],
                            qk_sb[:, (NH * DT + n * DT + d) * T + c0:
                                  (NH * DT + n * DT + d) * T + c0 + cn],
                            start=(d == 0), stop=(d == DT - 1))
                    if ci == len(schunks) - 1:  # diagonal tile: causal mask
                        dl = tau * P - c0
                        nc.vector.tensor_add(ps[:, dl:dl + P], ps[:, dl:dl + P],
                                             tri_sb[:, :])
                    acc = st_pool.tile([P, 1], f32, tag="acc")
                    nc.scalar.activation(p_sb[:, c0:c0 + cn], ps[:, :cn], Exp,
                                         accum_out=acc[:, :])
                    parts.append(acc)
                if len(parts) == 1:
                    tot = parts[0]
                else:
                    tot = st_pool.tile([P, 1], f32, tag="tot")
                    nc.vector.tensor_add(tot[:, :], parts[0][:, :], parts[1][:, :])
                rinv = st_pool.tile([P, 1], f32, tag="rinv")
                nc.vector.reciprocal(rinv[:, :], tot[:, :])
                nc.vector.tensor_scalar_mul(p_sb[:, :ext], p_sb[:, :ext],
                                            rinv[:, :])
                nc.sync.dma_start(ap_ap[b, n, tau * P:(tau + 1) * P, 0:ext],
                                  p_sb[:, :ext])
                nc.scalar.copy(pbf_sb[:, :ext], p_sb[:, :ext])
                for s in range(tau + 1):
                    nc.sync.dma_start(
                        pt_sb[:, s * T + tau * P: s * T + (tau + 1) * P],
                        pbf_sb[:, s * P:(s + 1) * P], transpose=True)
            # attnV for this head (accumulates over heads in PSUM)
            for d in range(DT):
                for bi, (t0, tn) in enumerate(tblocks):
                    ps = avps[d * len(tblocks) + bi]
                    last_s = (t0 + tn) // P - 1
                    for s in range(last_s + 1):
                        toff = max(s * P, t0)
                        nc.tensor.matmul(
                            ps[:, toff - t0: tn],
                            v_sb[:, s * DH + d * P: s * DH + (d + 1) * P],
                            pt_sb[:, s * T + toff: s * T + t0 + tn],
                            start=(n == 0 and s == 0),
                            stop=(n == NH - 1 and s == last_s))

        # ---- head-summed attn_vec -> bf16 ----
        av_sb = av_pool.tile([P, DT * T], bf16, tag="av")
        for d in range(DT):
            for bi, (t0, tn) in enumerate(tblocks):
                nc.scalar.copy(av_sb[:, d * T + t0: d * T + t0 + tn],
                               avps[d * len(tblocks) + bi][:, :tn])

        # ---- output projection ----
        for tau in range(NT):
            fin = fin_pool.tile([P, H], f32, tag="f")
            for (h0, hn) in _chunks(H, TBLK):
                ps = psA.tile([P, TBLK], f32, tag="ps")
                for d in range(DT):
                    nc.tensor.matmul(
                        ps[:, :hn],
                        av_sb[:, d * T + tau * P: d * T + (tau + 1) * P],
                        wo_sb[:, d * H + h0: d * H + h0 + hn],
                        start=(d == 0), stop=(d == DT - 1))
                nc.vector.tensor_copy(fin[:, h0:h0 + hn], ps[:, :hn])
            nc.sync.dma_start(out_ap[b, tau * P:(tau + 1) * P, :], fin[:, :])
    ctx.close()


def build(B=BPC, T=SEQ, H=HIDDEN, NH=N_HEAD, DH=D_HEAD):
    import concourse.bacc as bacc
    import concourse.mybir as mybir
    import concourse.tile as tile

    bf16 = mybir.dt.bfloat16
    f32 = mybir.dt.float32
    nc = bacc.Bacc("TRN2", target_bir_lowering=False, debug=False)
    xT = nc.dram_tensor("xT", [B, H, T], bf16, kind="ExternalInput").ap()
    wqk = nc.dram_tensor("w_qk", [H, 2 * NH * DH], bf16, kind="ExternalInput").ap()
    wv = nc.dram_tensor("w_v", [H, DH], bf16, kind="ExternalInput").ap()
    wo = nc.dram_tensor("w_out", [DH, H], bf16, kind="ExternalInput").ap()
    tri = nc.dram_tensor("tri", [P, P], f32, kind="ExternalInput").ap()
    out = nc.dram_tensor("out", [B, T, H], f32, kind="ExternalOutput").ap()
    ap_ = nc.dram_tensor("attn_prob", [B, NH, T, T], f32, kind="ExternalOutput").ap()
    with tile.TileContext(nc) as tc:
        _emit(tc, (xT, wqk, wv, wo, tri, out, ap_), B, T, H, NH, DH)
    nc.compile()
    return nc


def host_prep(x, w_qkv, w_out, NH=N_HEAD, DH=D_HEAD):
    import ml_dtypes
    bf16 = ml_dtypes.bfloat16
    xT = np.ascontiguousarray(x.transpose(0, 2, 1)).astype(bf16)
    scale = np.float32(DH) ** np.float32(-0.5)
    wqk = np.concatenate([w_qkv[:, :NH * DH] * scale,
                          w_qkv[:, NH * DH:2 * NH * DH]], axis=1).astype(bf16)
    wv = np.ascontiguousarray(w_qkv[:, 2 * NH * DH:]).astype(bf16)
    wo = (w_out * (np.float32(1.0) / np.float32(NH))).astype(bf16)
    r = np.arange(P)
    tri = np.where(r[None, :] > r[:, None], np.float32(-1e9),
                   np.float32(0.0)).astype(np.float32)
    return xT, wqk, wv, wo, tri


def kernel(x, w_qkv, w_out):
    from concourse.bass_utils import run_bass_kernel_spmd

    assert x.shape == (BATCH, SEQ, HIDDEN), x.shape
    if "nc" not in _compiled:
        _compiled["nc"] = build()
    nc = _compiled["nc"]
    xT, wqk, wv, wo, tri = host_prep(x, w_qkv, w_out)
    in_maps = [{"xT": xT[c * BPC:(c + 1) * BPC], "w_qk": wqk, "w_v": wv,
                "w_out": wo, "tri": tri} for c in range(N_CORES)]
    res = run_bass_kernel_spmd(nc, in_maps, core_ids=list(range(N_CORES)))
    out = np.concatenate([res.results[c]["out"] for c in range(N_CORES)], axis=0)
    attn_prob = np.concatenate([res.results[c]["attn_prob"]
                                for c in range(N_CORES)], axis=0)
    return out.astype(np.float32), attn_prob.astype(np.float32)


# revision 7
# speedup vs baseline: 1.0346x; 1.0346x over previous
"""Trainium2 Bass kernel: InterpretableMultiHeadAttention (shared-V MHA).

Contract: kernel(x, w_qkv, w_out) -> (out, attn_prob), full (unsharded) numpy
in/out. Internally: data-parallel over batch across 8 NeuronCores (2 batches
per core), bf16 compute on TensorE, f32 softmax/outputs.

Per-core dataflow (B=2 local batches, T=1024, H=1024, NH=4, DH=256):
  - host pre-transposes x to xT[b, h, t] bf16 (avoids on-chip transposes;
    q-scale 1/sqrt(DH) folded into w_q, head-mean 1/NH folded into w_out)
  - qkT[f, t] = w_qk.T @ x.T   (lhsT = w_qk tiles, rhs = xT tiles)
  - v[s, d]   = x @ w_v        (lhsT = xT tiles,   rhs = w_v tiles)
  - per head: scores[t, s] psum -> +tri mask on diag tile -> Exp (ACT, with
    row-sum accum) -> reciprocal -> normalize -> f32 DMA to attn_prob
    (upper-triangular tiles left unwritten; PJRT pre-zeros output buffers)
    -> bf16 cast -> per-tile SBUF->SBUF DMA transpose into pT[s, t]
  - attnV: psum[d, t] += v[s,:].T @ pT[s, t] accumulated over s AND heads
    (head-sum free in PSUM; suffix-N per s-tile skips masked wedge)
  - out[t, :] = (sum_heads attn_vec).T @ w_out (1/NH prefolded)
"""

import numpy as np

N_HEAD = 4
HIDDEN = 1024
D_HEAD = HIDDEN // N_HEAD  # 256
SEQ = 1024
BATCH = 16
N_CORES = 8
BPC = BATCH // N_CORES  # batches per core

P = 128
TBLK = 512  # matmul moving free-dim block

_compiled = {}

# timing-ablation flags (timing experiments only; break correctness)
ABL = set()


def _chunks(total, size):
    out = []
    o = 0
    while o < total:
        out.append((o, min(size, total - o)))
        o += size
    return out


def _emit(tc, aps, B, T, H, NH, DH):
    import concourse.mybir as mybir

    nc = tc.nc
    bf16 = mybir.dt.bfloat16
    f32 = mybir.dt.float32
    Exp = mybir.ActivationFunctionType.Exp

    xT_ap, wqk_ap, wv_ap, wo_ap, tri_ap, out_ap, ap_ap = aps
    HT = H // P            # h-tiles over hidden
    FQK = 2 * NH * DH      # q+k features
    FT = FQK // P          # qk f-tiles
    NT = T // P            # t/s tiles
    DT = DH // P           # d-tiles per head
    tblocks = _chunks(T, TBLK)

    from contextlib import ExitStack
    ctx = ExitStack()
    const = ctx.enter_context(tc.tile_pool(name="const", bufs=1))
    xt_pool = ctx.enter_context(tc.tile_pool(name="xt", bufs=2))
    qk_pool = ctx.enter_context(tc.tile_pool(name="qkp", bufs=1))
    v_pool = ctx.enter_context(tc.tile_pool(name="vp", bufs=2))
    pt_pool = ctx.enter_context(tc.tile_pool(name="pt", bufs=2))
    p_pool = ctx.enter_context(tc.tile_pool(name="pp", bufs=5))
    pbf_pool = ctx.enter_context(tc.tile_pool(name="pbf", bufs=2))
    av_pool = ctx.enter_context(tc.tile_pool(name="avs", bufs=2))
    fin_pool = ctx.enter_context(tc.tile_pool(name="fin", bufs=3))
    st_pool = ctx.enter_context(tc.tile_pool(name="st", bufs=8))
    psA = ctx.enter_context(tc.tile_pool(name="psA", bufs=4, space="PSUM"))
    psB = ctx.enter_context(tc.tile_pool(name="psB", bufs=4, space="PSUM"))
    dma_rr = [nc.gpsimd, nc.sync, nc.scalar]
    rr = [0]
    trr = [0]

    # ---- constants (loaded once) ----
    w_sb = const.tile([P, HT * FQK], bf16, tag="wqk")
    for h in range(HT):
        nc.sync.dma_start(w_sb[:, h * FQK:(h + 1) * FQK], wqk_ap[h * P:(h + 1) * P, :])
    wv_sb = const.tile([P, HT * DH], bf16, tag="wv")
    for h in range(HT):
        nc.sync.dma_start(wv_sb[:, h * DH:(h + 1) * DH], wv_ap[h * P:(h + 1) * P, :])
    wo_sb = const.tile([P, DT * H], bf16, tag="wo")
    for d in range(DT):
        nc.sync.dma_start(wo_sb[:, d * H:(d + 1) * H], wo_ap[d * P:(d + 1) * P, :])
    tri_sb = const.tile([P, P], f32, tag="tri")
    nc.sync.dma_start(tri_sb[:], tri_ap[:, :])

    for b in range(B):
        # ---- load xT (bf16) ----
        xt_sb = xt_pool.tile([P, HT * T], bf16, tag="xt")
        for h in range(HT):
            nc.sync.dma_start(xt_sb[:, h * T:(h + 1) * T],
                              xT_ap[b, h * P:(h + 1) * P, :])

        # ---- QK projection: qkT[f, t] ----
        qk_sb = qk_pool.tile([P, FT * T], bf16, tag="qk")
        for f in range(FT):
            for (t0, tn) in tblocks:
                ps = psA.tile([P, TBLK], f32, tag="ps")
                for h in range(HT):
                    nc.tensor.matmul(
                        ps[:, :tn],
                        w_sb[:, h * FQK + f * P: h * FQK + (f + 1) * P],
                        xt_sb[:, h * T + t0: h * T + t0 + tn],
                        start=(h == 0), stop=(h == HT - 1))
                nc.vector.tensor_copy(qk_sb[:, f * T + t0: f * T + t0 + tn],
                                      ps[:, :tn])

        # ---- V projection: v[s, d] ----
        v_sb = v_pool.tile([P, NT * DH], bf16, tag="v")
        for s in range(NT):
            ps = psA.tile([P, TBLK], f32, tag="ps")
            for h in range(HT):
                nc.tensor.matmul(
                    ps[:, :DH],
                    xt_sb[:, h * T + s * P: h * T + (s + 1) * P],
                    wv_sb[:, h * DH:(h + 1) * DH],
                    start=(h == 0), stop=(h == HT - 1))
            nc.vector.tensor_copy(v_sb[:, s * DH:(s + 1) * DH], ps[:, :DH])

        # ---- attention ----
        avps = [psB.tile([P, TBLK], f32, tag="av", name=f"avp{i}")
                for i in range(DT * len(tblocks))]
        for n in range(NH):
            pt_sb = pt_pool.tile([P, NT * T], bf16, tag="pt")
            for tau in range(NT):
                ext = (tau + 1) * P
                p_sb = p_pool.tile([P, T], f32, tag="p")
                pbf_sb = pbf_pool.tile([P, T], bf16, tag="pb")
                schunks = _chunks(ext, TBLK)
                parts = []
                for ci, (c0, cn) in enumerate(schunks):
                    ps = psA.tile([P, TBLK], f32, tag="ps")
                    for d in range(DT):
                        nc.tensor.matmul(
                            ps[:, :cn],
                            qk_sb[:, (n * DT + d) * T + tau * P:
                                  (n * DT + d) * T + (tau + 1) * P],
                            qk_sb[:, (NH * DT + n * DT + d) * T + c0:
                                  (NH * DT + n * DT + d) * T + c0 + cn],
                            start=(d == 0), stop=(d == DT - 1))
                    if ci == len(schunks) - 1:  # diagonal tile: causal mask
                        dl = tau * P - c0
                        nc.vector.tensor_add(ps[:, dl:dl + P], ps[:, dl:dl + P],
                                             tri_sb[:, :])
                    acc = st_pool.tile([P, 1], f32, tag="acc")
                    nc.scalar.activation(p_sb[:, c0:c0 + cn], ps[:, :cn], Exp,
                                         accum_out=acc[:, :])
                    parts.append(acc)
                if len(parts) == 1:
                    tot = parts[0]
                else:
                    tot = st_pool.tile([P, 1], f32, tag="tot")
                    nc.vector.tensor_add(tot[:, :], parts[0][:, :], parts[1][:, :])
                rinv = st_pool.tile([P, 1], f32, tag="rinv")
                nc.vector.reciprocal(rinv[:, :], tot[:, :])
                for (c0, cn) in schunks:
                    if "norm" not in ABL:
                        nc.vector.tensor_scalar_mul(p_sb[:, c0:c0 + cn],
                                                    p_sb[:, c0:c0 + cn],
                                                    rinv[:, :])
                    if "apdma" not in ABL:
                        eng = dma_rr[rr[0] % len(dma_rr)]
                        rr[0] += 1
                        eng.dma_start(
                            ap_ap[b, n, tau * P:(tau + 1) * P, c0:c0 + cn],
                            p_sb[:, c0:c0 + cn])
                if "cast" not in ABL:
                    nc.scalar.copy(pbf_sb[:, :ext], p_sb[:, :ext])
                if "trans" not in ABL:
                    pt3 = pt_sb.rearrange("p (s t) -> p s t", t=T)
                    teng = (nc.sync, nc.scalar)[trr[0] % 2]
                    trr[0] += 1
                    teng.dma_start(
                        pt3[:, 0:tau + 1, tau * P:(tau + 1) * P],
                        pbf_sb[:, :ext], transpose=True)
            # attnV for this head (accumulates over heads in PSUM)
            for d in range(DT):
                for bi, (t0, tn) in enumerate(tblocks):
                    ps = avps[d * len(tblocks) + bi]
                    last_s = (t0 + tn) // P - 1
                    for s in range(last_s + 1):
                        toff = max(s * P, t0)
                        nc.tensor.matmul(
                            ps[:, toff - t0: tn],
                            v_sb[:, s * DH + d * P: s * DH + (d + 1) * P],
                            pt_sb[:, s * T + toff: s * T + t0 + tn],
                            start=(n == 0 and s == 0),
                            stop=(n == NH - 1 and s == last_s))

        # ---- head-summed attn_vec -> bf16 ----
        av_sb = av_pool.tile([P, DT * T], bf16, tag="av")
        for d in range(DT):
            for bi, (t0, tn) in enumerate(tblocks):
                nc.scalar.copy(av_sb[:, d * T + t0: d * T + t0 + tn],
                               avps[d * len(tblocks) + bi][:, :tn])

        # ---- output projection ----
        for tau in range(NT):
            fin = fin_pool.tile([P, H], f32, tag="f")
            for (h0, hn) in _chunks(H, TBLK):
                ps = psB.tile([P, TBLK], f32, tag="av", name="finps")
                for d in range(DT):
                    nc.tensor.matmul(
                        ps[:, :hn],
                        av_sb[:, d * T + tau * P: d * T + (tau + 1) * P],
                        wo_sb[:, d * H + h0: d * H + h0 + hn],
                        start=(d == 0), stop=(d == DT - 1))
                nc.vector.tensor_copy(fin[:, h0:h0 + hn], ps[:, :hn])
            nc.gpsimd.dma_start(out_ap[b, tau * P:(tau + 1) * P, :], fin[:, :])
    ctx.close()


def build(B=BPC, T=SEQ, H=HIDDEN, NH=N_HEAD, DH=D_HEAD):
    import concourse.bacc as bacc
    import concourse.mybir as mybir
    import concourse.tile as tile

    bf16 = mybir.dt.bfloat16
    f32 = mybir.dt.float32
    nc = bacc.Bacc("TRN2", target_bir_lowering=False, debug=False)
    xT = nc.dram_tensor("xT", [B, H, T], bf16, kind="ExternalInput").ap()
    wqk = nc.dram_tensor("w_qk", [H, 2 * NH * DH], bf16, kind="ExternalInput").ap()
    wv = nc.dram_tensor("w_v", [H, DH], bf16, kind="ExternalInput").ap()
    wo = nc.dram_tensor("w_out", [DH, H], bf16, kind="ExternalInput").ap()
    tri = nc.dram_tensor("tri", [P, P], f32, kind="ExternalInput").ap()
    out = nc.dram_tensor("out", [B, T, H], f32, kind="ExternalOutput").ap()
    ap_ = nc.dram_tensor("attn_prob", [B, NH, T, T], f32, kind="ExternalOutput").ap()
    with tile.TileContext(nc) as tc:
        _emit(tc, (xT, wqk, wv, wo, tri, out, ap_), B, T, H, NH, DH)
    nc.compile()
    return nc


def host_prep(x, w_qkv, w_out, NH=N_HEAD, DH=D_HEAD):
    import ml_dtypes
    bf16 = ml_dtypes.bfloat16
    xT = np.ascontiguousarray(x.transpose(0, 2, 1)).astype(bf16)
    scale = np.float32(DH) ** np.float32(-0.5)
    wqk = np.concatenate([w_qkv[:, :NH * DH] * scale,
                          w_qkv[:, NH * DH:2 * NH * DH]], axis=1).astype(bf16)
    wv = np.ascontiguousarray(w_qkv[:, 2 * NH * DH:]).astype(bf16)
    wo = (w_out * (np.float32(1.0) / np.float32(NH))).astype(bf16)
    r = np.arange(P)
    tri = np.where(r[None, :] > r[:, None], np.float32(-1e9),
                   np.float32(0.0)).astype(np.float32)
    return xT, wqk, wv, wo, tri


def kernel(x, w_qkv, w_out):
    from concourse.bass_utils import run_bass_kernel_spmd

    assert x.shape == (BATCH, SEQ, HIDDEN), x.shape
    if "nc" not in _compiled:
        _compiled["nc"] = build()
    nc = _compiled["nc"]
    xT, wqk, wv, wo, tri = host_prep(x, w_qkv, w_out)
    in_maps = [{"xT": xT[c * BPC:(c + 1) * BPC], "w_qk": wqk, "w_v": wv,
                "w_out": wo, "tri": tri} for c in range(N_CORES)]
    res = run_bass_kernel_spmd(nc, in_maps, core_ids=list(range(N_CORES)))
    out = np.concatenate([res.results[c]["out"] for c in range(N_CORES)], axis=0)
    attn_prob = np.concatenate([res.results[c]["attn_prob"]
                                for c in range(N_CORES)], axis=0)
    return out.astype(np.float32), attn_prob.astype(np.float32)
